# revision 24
# baseline (speedup 1.0000x reference)
"""CompressedLinear (int8-dequant linear) Trainium2 kernel.

Math: out[b,s,o] = sum_d x[b,s,d] * (weight_int8[o,d] * scale[o]) + bias[o]
with x: [4, 2048, 4096] f32, weight_int8: [4096, 4096] int32 (int8 values),
scale/bias: [4096] f32.

Sharding: data-parallel over tokens. The 8192 tokens are split 1024/core
across 8 cores; each core computes its token shard against the full weight.
Per-core HBM traffic is x_shard(fp16 8MB) + W(fp16 32MB) + out(f32 16MB)
= 56MB, well under the ~440us of PE time for the 34 GFLOP/core GEMM, so
the kernel is compute-bound (vs ~148MB/core for output-dim tensor
parallelism, which would be memory-bound). No collectives; shards concat
on host.

On-device layout: out^T[dout, tok] = W_int8 @ x_shard^T, computed as
  psum[dout_p=128, tok_f=512] += wT_tile[din_k=128, dout=128].T
                                 @ xT_tile[din_k=128, tok=512]
accumulated over 32 k-tiles in fp32 PSUM. The GEMM runs in fp16: the int8
weight values are exact in fp16, x rounds at 2^-11 (rel err ~3e-4), and
fp16 products are exact in the fp32 accumulator. d_out lands on the
partition dim, so the per-output-row dequant scale and bias are fused into
the PSUM->SBUF eviction as a per-partition scalar.activation(Identity):
out = psum * scale + bias.

Raw bass (not Tile): this toolchain's walrus rejects DMA descriptors with
more than one semaphore wait, which Tile's auto-sem pass emits for any
slot-reusing DMA stream. Here all waits are standalone sequencer
instructions; every DMA carries zero waits and one completion increment.

Pipeline (per core):
  SP (sync)   : scale/bias, then the even-k half of each W block — one
                512KB partition-major DMA per block, triple-buffered
                (WSETS=3), gated by s_pe so a W set is only overwritten
                after its consumer block finished.
  ACT (scalar): the odd-k half of each W block (second HWDGE ring), plus
                per block two PSUM->SBUF evictions fused with
                scale*psum+bias into a 4-deep output rotation.
  PE (tensor) : per block: 64 matmuls (32 k-tiles x 2 token tiles) into
                two PSUM banks; bank pair alternates per block, gated on
                s_act (eviction done). Incs s_pe once per block.
  POOL(gpsimd): x shard load (resident, 8MB), then per eviction one
                [128,512] f32 store to outT.

DMA-completion gating is made exact under HW-DGE queue fan-out (a large
DMA's +16 completion increment can arrive piecemeal and out of order
across physical queues):
  - W: one rotation sem per (ring, set); s_pe issue-gating leaves at most
    one outstanding DMA per sem, so its count is unambiguous.
  - x / stores: the issuing engine self-waits on its own cumulative count
    at group boundaries (exact: nothing beyond the checkpoint is issued)
    and relays completion via engine sems s_xg / s_og.
"""

from contextlib import ExitStack

import numpy as np

import concourse.bass as bass
from concourse import mybir
from concourse.bass_utils import run_bass_kernel_spmd

B, S, D_IN, D_OUT = 4, 2048, 4096, 4096
N_CORES = 8
TOKENS = B * S  # 8192
TOK = TOKENS // N_CORES  # 1024 tokens per core
P = 128
KT = D_IN // P  # 32 contraction tiles
NBLK = D_OUT // P  # 32 output-row blocks
TFREE = 512  # matmul moving free dim / one PSUM bank of f32
TT = TOK // TFREE  # 2 token tiles per core
OROT = 4  # output-tile rotation depth
WSETS = 3  # weight double^H triple buffering (one 128-row block per set)
WHALF = KT // 2 * P  # free-dim elems of one parity half of a W set

_CACHE = {}
TRACE = False
LAST_RESULT = None


def _build_program(reps=1):
    """Build the kernel program. reps>1 wraps the whole pipeline in an
    all-engine Fori loop — used only for benchmarking (amortizes the
    ~80ms axon RPC floor over many executions)."""
    key = ("nc", reps)
    if key in _CACHE:
        return _CACHE[key]
    nc = bass.Bass()
    f16, f32 = mybir.dt.float16, mybir.dt.float32
    xT = nc.dram_tensor("xT", [D_IN, TOK], f16, kind="ExternalInput")
    # [blk][parity][din%128 p][j][dout 128]: tile k=2j+par of block blk.
    # Each (blk, parity) half is 512KB contiguous, partition-major: every
    # SBUF partition reads one 4KB contiguous DRAM run -> one efficient
    # DMA per ring per block.
    w = nc.dram_tensor("w", [NBLK, 2, P, KT // 2, P], f16, kind="ExternalInput")
    scaleT = nc.dram_tensor("scaleT", [P, NBLK], f32, kind="ExternalInput")
    biasT = nc.dram_tensor("biasT", [P, NBLK], f32, kind="ExternalInput")
    outT = nc.dram_tensor("outT", [D_OUT, TOK], f32, kind="ExternalOutput")

    with (
        nc.sbuf_tensor([P, KT * TOK], f16) as x_sb,  # 64KB/partition
        nc.sbuf_tensor([P, WSETS * KT * P], f16) as w_sb,  # 24KB/partition
        nc.sbuf_tensor([P, NBLK], f32) as sc_sb,
        nc.sbuf_tensor([P, NBLK], f32) as bi_sb,
        nc.sbuf_tensor([P, OROT * TFREE], f32) as ot_sb,  # 8KB/partition
        nc.psum_tensor([P, 4 * TFREE], f32) as ps,  # 4 banks
        ExitStack() as stack,
    ):
        names = [
            "s_x", "s_pe", "s_act", "s_out", "s_c",
            "s_wa0", "s_wa1", "s_wa2", "s_wb0", "s_wb1", "s_wb2",
            "s_xg", "s_og",
        ]
        sem = {n: stack.enter_context(nc.semaphore(n)) for n in names}
        s_x, s_pe, s_act = sem["s_x"], sem["s_pe"], sem["s_act"]
        s_out, s_c = sem["s_out"], sem["s_c"]
        s_xg, s_og = sem["s_xg"], sem["s_og"]
        s_wr = [
            [sem["s_wa0"], sem["s_wa1"], sem["s_wa2"]],
            [sem["s_wb0"], sem["s_wb1"], sem["s_wb2"]],
        ]
        if reps > 1:
            stack.enter_context(nc.Fori(0, reps))
        # Re-execution safety: sems are NOT zeroed by allocation. Clear
        # them (and drain any DMA state bound to them) before any engine
        # uses them; the barrier keeps other engines out until done.
        nums = sorted(s.num for s in sem.values())
        assert nums[-1] - nums[0] == len(nums) - 1, nums
        rng = range(nums[0], nums[-1] + 1)
        nc.gpsimd.dma_reset(rng)
        nc.gpsimd.sem_clear(rng)
        nc.all_engine_barrier()
        block = stack.enter_context(nc.Block())

        def w_dma(eng, blk, par):
            base = (blk % WSETS) * KT * P + par * WHALF
            eng.dma_start(
                out=w_sb[:, base : base + WHALF],
                in_=w[blk, par].rearrange("p j o -> p (j o)"),
            ).then_inc(s_wr[par][blk % WSETS], 16)

        # Weight stream is ~32MB/core: split across BOTH HWDGE rings
        # (SP parity 0, ACT parity 1), one 512KB DMA per ring per block.

        @block.sync
        def _(sync: bass.BassEngine):
            sync.dma_start(out=sc_sb[:], in_=scaleT[:]).then_inc(s_c, 16)
            sync.dma_start(out=bi_sb[:], in_=biasT[:]).then_inc(s_c, 16)
            for blk in range(WSETS):
                w_dma(sync, blk, 0)
            for blk in range(NBLK - WSETS):
                # set (blk+WSETS)%WSETS == blk%WSETS free once blk finished
                sync.wait_ge(s_pe, blk + 1)
                w_dma(sync, blk + WSETS, 0)

        @block.tensor
        def _(tensor: bass.BassEngine):
            for blk in range(NBLK):
                base = (blk % WSETS) * KT * P
                pbase = (blk % 2) * 2 * TFREE
                if blk >= 2:
                    # PSUM bank pair free once blk-2's evictions are done
                    tensor.wait_ge(s_act, 2 * blk - 2)
                tensor.wait_ge(s_wr[0][blk % WSETS], 16 * (blk // WSETS + 1))
                tensor.wait_ge(s_wr[1][blk % WSETS], 16 * (blk // WSETS + 1))
                for k in range(KT):
                    if blk == 0:
                        tensor.wait_ge(s_xg, k // 4 + 1)
                    lw = base + (k % 2) * WHALF + (k // 2) * P
                    for ti in range(TT):
                        mm = nc.tensor.matmul(
                            ps[:, pbase + ti * TFREE : pbase + (ti + 1) * TFREE],
                            lhsT=w_sb[:, lw : lw + P],
                            rhs=x_sb[:, k * TOK + ti * TFREE : k * TOK + (ti + 1) * TFREE],
                            start=(k == 0),
                            stop=(k == KT - 1),
                        )
                mm.then_inc(s_pe, 1)

        @block.scalar
        def _(scalar: bass.BassEngine):
            scalar.wait_ge(s_c, 32)  # scale/bias loaded
            for blk in range(WSETS):
                w_dma(scalar, blk, 1)
            for blk in range(NBLK):
                scalar.wait_ge(s_pe, blk + 1)
                pbase = (blk % 2) * 2 * TFREE
                for ti in range(TT):
                    g = 2 * blk + ti
                    if g >= OROT:
                        scalar.wait_ge(s_og, (g - OROT) // 2 + 1)
                    slot = (g % OROT) * TFREE
                    nc.scalar.activation(
                        ot_sb[:, slot : slot + TFREE],
                        ps[:, pbase + ti * TFREE : pbase + (ti + 1) * TFREE],
                        mybir.ActivationFunctionType.Identity,
                        bias=bi_sb[:, blk : blk + 1],
                        scale=sc_sb[:, blk : blk + 1],
                    ).then_inc(s_act, 1)
                if blk + WSETS < NBLK:
                    # s_pe >= blk+1 already guarantees the target set is free
                    w_dma(scalar, blk + WSETS, 1)

        @block.gpsimd
        def _(gpsimd: bass.BassEngine):
            for k in range(KT):
                gpsimd.dma_start(
                    out=x_sb[:, k * TOK : (k + 1) * TOK],
                    in_=xT[k * P : (k + 1) * P, :],
                ).then_inc(s_x, 16)
                if k % 4 == 3:
                    gpsimd.wait_ge(s_x, 16 * (k + 1))
                    gpsimd.sem_inc(s_xg, 1)
            for blk in range(NBLK):
                for ti in range(TT):
                    g = 2 * blk + ti
                    gpsimd.wait_ge(s_act, g + 1)
                    slot = (g % OROT) * TFREE
                    gpsimd.dma_start(
                        out=outT[
                            blk * P : (blk + 1) * P, ti * TFREE : (ti + 1) * TFREE
                        ],
                        in_=ot_sb[:, slot : slot + TFREE],
                    ).then_inc(s_out, 16)
                    if g % 2 == 1:
                        gpsimd.wait_ge(s_out, 16 * (g + 1))
                        gpsimd.sem_inc(s_og, 1)
            gpsimd.wait_ge(s_out, 16 * NBLK * TT)  # all stores landed

    _CACHE[key] = nc
    return nc


def prepare_inputs(x, weight_int8, scale, bias):
    x2 = np.asarray(x, dtype=np.float32).reshape(TOKENS, D_IN)
    w16 = np.asarray(weight_int8).astype(np.float16)  # int8 values: exact
    # [blk][par][p][j][o] with k = 2j + par: partition-major halves
    t = w16.T.reshape(KT, P, NBLK, P)  # [k, p, blk, o]
    wtiled = np.ascontiguousarray(
        t.transpose(2, 0, 1, 3)              # [blk, k, p, o]
        .reshape(NBLK, KT // 2, 2, P, P)     # [blk, j, par, p, o]
        .transpose(0, 2, 3, 1, 4)            # [blk, par, p, j, o]
    )
    scaleT = np.ascontiguousarray(
        np.asarray(scale, dtype=np.float32).reshape(NBLK, P).T
    )
    biasT = np.ascontiguousarray(
        np.asarray(bias, dtype=np.float32).reshape(NBLK, P).T
    )
    in_maps = []
    for i in range(N_CORES):
        xTi = x2[i * TOK : (i + 1) * TOK, :].T.astype(np.float16)
        in_maps.append({"xT": xTi, "w": wtiled, "scaleT": scaleT, "biasT": biasT})
    return in_maps


def kernel(x, weight_int8, scale, bias):
    global LAST_RESULT
    nc = _build_program()
    in_maps = prepare_inputs(x, weight_int8, scale, bias)
    LAST_RESULT = run_bass_kernel_spmd(
        nc, in_maps, core_ids=list(range(N_CORES)), trace=TRACE
    )
    outT = np.concatenate([r["outT"] for r in LAST_RESULT.results], axis=1)
    return np.ascontiguousarray(outT.T).reshape(B, S, D_OUT).astype(np.float32)
